# revision 31
# baseline (speedup 1.0000x reference)
"""Trainium2 Bass kernel for nn_CenterNeighAtt (gnn_message_passing).

Reference computation (E=512, F=256, R=4):
    adj_agg   = softmax(adj.sum(0), axis=1)                       # [E,E]
    t[i,j,f]  = leaky_relu(adj_agg[i,j] * h[i,f] * h[j,f], 0.2)   # [E,E,F]
    att       = softmax(t, axis=0)  (over i)                      # [E,E,F]
    scores    = einsum('ijf,f->ij', att, w) + b
    alpha     = softmax(scores.sum(0)[None,:,None], axis=1)       # [1,E,1]
    h_prime   = einsum('ijf,jf->if', att, h)                      # [E,F]
    returns (elu(h_prime), alpha)

Key identities used:
  * sum_i att[i,j,f] == 1 exactly, so scores.sum(0)[j] = sum(w) + E*b is a
    constant vector -> alpha == uniform 1/E for ANY w, b. (Float deviation of
    the reference from uniform is ~1e-6 relative; far below tolerance.)
  * The softmax over i never needs the max-subtraction here: |t| <= ~2
    (adj_agg in (0, ~0.1), |h| <= ~5), so exp() is exact-safe in fp32/bf16.

Sharding: the j axis (neighbor axis) is sharded across the 8 NeuronCores,
64 j's per core. Each core computes, for its j-shard, the full [F, E(i)]
contribution sum_j att[i,j,f] * h[j,f] with NO collectives: softmax over i
is local because i lives entirely in the free axis. The host sums the 8
partial [F,E] outputs, transposes, applies elu.

Per-core layout: partitions = f (2 blocks of 128 paired in the free dim),
free = i (512), loop j. Per j (b = f-block index):
    DMA : Bt       = adj_agg[:,j] row broadcast to [128,E] bf16 (stride-0
                     partition source; DMA engines are otherwise idle)
    DVE : HH[b]    = ht2[b] * h[j,fb]              tensor_scalar, bf16 4x
    DVE : W2       = HH * Bt                       [128,2E] bf16 TT 2x
    DVE/ACT (rotated): L2 = leaky_relu(W2)         STT pair / Prelu pair
    ACT : u[b]     = exp(L2[b]), s[b] = rowsum     accum_out fused, bf16 out
    DVE : g2       = (1/s) * h[j,f]                tiny [128,2] ops
    DVE : diag_b   = I * g2[:,b]                   [128,128] bf16 diag matrix
    PE  : HP[b]   += diag_b.T @ u[b]               PSUM accumulation over j
j's are processed in pairs ([128, 2(j), 2(b), E] tiles) to halve per-op
overhead on the big passes. The TensorEngine's only job is the h_prime
accumulation (PSUM accumulate), the ScalarEngine does exp (+fused row-sums)
and two fifths of the leaky-relus, the VectorEngine does the bf16 product
chain and the rest.
"""

import time

import numpy as np

E, F, R, NCORES = 512, 256, 4, 8
JPC = E // NCORES  # j's per core
NEG_SLOPE = 0.2
# leaky relu runs on DVE for 3 of every 5 j-pairs, ACT for the rest
# (measured balance point of the two engines)

_CACHE = {}
LAST_RESULTS = None      # BassKernelResults of the last run (for test harness)
LAST_RUN_WALL_S = None   # wall time of the last run_bass_kernel_spmd call


def _build_nc():
    import concourse.bacc as bacc
    import concourse.mybir as mybir
    from concourse import tile

    f32 = mybir.dt.float32
    bf16 = mybir.dt.bfloat16
    Alu = mybir.AluOpType
    Act = mybir.ActivationFunctionType

    nc = bacc.Bacc(
        "TRN2", target_bir_lowering=False, debug=False, num_devices=NCORES
    )
    # ht2[p, b, i]  = h[i, b*128+p]   (both f-blocks side by side in free)
    ht2_d = nc.dram_tensor("ht2", [128, 2, E], bf16, kind="ExternalInput")
    # htc2[p, jl, b] = h[jg, b*128+p] (this core's j columns, per f-block)
    htc2_d = nc.dram_tensor("htc2", [128, JPC, 2], f32, kind="ExternalInput")
    # adjtb[jl, i] = adj_agg[i, jg] (bf16 rows, DMA-broadcast per j)
    adjtb_d = nc.dram_tensor("adjtb", [JPC, E], bf16, kind="ExternalInput")
    ident_d = nc.dram_tensor("ident", [128, 128], bf16, kind="ExternalInput")
    hp_d = nc.dram_tensor("hp", [128, 2, E], f32, kind="ExternalOutput")

    with tile.TileContext(nc) as tc:
        with (
            tc.tile_pool(name="const", bufs=1) as cpool,
            tc.tile_pool(name="bcast", bufs=6) as bpool,
            tc.tile_pool(name="wrk", bufs=4) as wpool,
            tc.tile_pool(name="lu", bufs=4) as lupool,
            tc.tile_pool(name="small", bufs=10) as spool,
            tc.tile_pool(name="psH", bufs=1, space="PSUM") as hpool,
        ):
            ht2 = cpool.tile([128, 2, E], bf16, tag="ht2")
            nc.sync.dma_start(ht2[:], ht2_d[:])
            htc2 = cpool.tile([128, JPC, 2], f32, tag="htc2")
            nc.sync.dma_start(htc2[:], htc2_d[:])
            ident = cpool.tile([128, 128], bf16, tag="ident")
            nc.sync.dma_start(ident[:], ident_d[:])

            HP = []
            for b in range(2):
                hp_acc = hpool.tile([128, E], f32, tag=f"hp{b}")
                HP.append(hp_acc)

            # process j's in pairs: [128, 2(j), 2(b), E] tiles halve the
            # per-op overhead tax on the big DVE/ACT passes
            NJP = JPC // 2
            for jp in range(NJP):
                j0 = 2 * jp
                bt2 = bpool.tile([128, 2, E], bf16, tag="bt2")
                nc.sync.dma_start(
                    bt2[:],
                    adjtb_d[j0:j0 + 2, :]
                    .rearrange("(o j) i -> o j i", o=1)
                    .broadcast_to((128, 2, E)),
                )

                hh4 = wpool.tile([128, 2, 2, E], bf16, tag="hh4")
                for jj in range(2):
                    for b in range(2):
                        nc.vector.tensor_scalar(
                            hh4[:, jj, b, :], ht2[:, b, :],
                            htc2[:, j0 + jj, b:b + 1], None, op0=Alu.mult,
                        )

                w4 = wpool.tile([128, 2, 2, E], bf16, tag="w4")
                nc.vector.tensor_tensor(
                    w4[:], hh4[:],
                    bt2[:].rearrange("p j (o i) -> p j o i", o=1)
                          .broadcast_to((128, 2, 2, E)),
                    op=Alu.mult,
                )

                # leaky relu, alternating DVE/ACT pairs to balance budgets
                l4 = wpool.tile([128, 2, 2, E], bf16, tag="l4")
                if jp % 5 < 3:
                    nc.vector.scalar_tensor_tensor(
                        l4[:], w4[:], NEG_SLOPE, w4[:],
                        op0=Alu.mult, op1=Alu.max,
                    )
                else:
                    nc.scalar.activation(l4[:], w4[:], Act.Prelu,
                                         alpha=NEG_SLOPE)

                # exp per (j, b) with the row-sum fused into the activation
                u4 = lupool.tile([128, 2, 2, E], bf16, tag="u4")
                s4 = spool.tile([128, 2, 2], f32, tag="s4")
                for jj in range(2):
                    for b in range(2):
                        nc.scalar.activation(
                            u4[:, jj, b, :], l4[:, jj, b, :], Act.Exp,
                            accum_out=s4[:, jj, b:b + 1],
                        )

                r4 = spool.tile([128, 2, 2], f32, tag="r4")
                nc.vector.reciprocal(r4[:], s4[:])
                g4 = spool.tile([128, 2, 2], f32, tag="g4")
                nc.vector.tensor_tensor(
                    g4[:], r4[:], htc2[:, j0:j0 + 2, :], op=Alu.mult
                )

                for jj in range(2):
                    for b in range(2):
                        diag = spool.tile([128, 128], bf16, tag=f"diag{b}")
                        nc.vector.tensor_scalar(
                            diag[:], ident[:], g4[:, jj, b:b + 1], None,
                            op0=Alu.mult,
                        )
                        nc.tensor.matmul(
                            HP[b][:],
                            diag[:],
                            u4[:, jj, b, :],
                            start=(jp == 0 and jj == 0),
                            stop=(jp == NJP - 1 and jj == 1),
                        )

            for b in range(2):
                hp_sb = wpool.tile([128, E], f32, tag=f"hpsb{b}")
                nc.vector.tensor_copy(hp_sb[:], HP[b][:])
                nc.sync.dma_start(hp_d[:, b, :], hp_sb[:])

    nc.compile()
    return nc


def get_nc():
    if "nc" not in _CACHE:
        _CACHE["nc"] = _build_nc()
    return _CACHE["nc"]


def _softmax_rows_f32(x):
    x = np.asarray(x, dtype=np.float32)
    m = x.max(axis=1, keepdims=True)
    e = np.exp(x - m, dtype=np.float32)
    return e / e.sum(axis=1, keepdims=True, dtype=np.float32)


def make_in_maps(h, adj):
    """Shard inputs for the 8 cores (host-side slicing/layout only)."""
    import ml_dtypes

    bf16 = ml_dtypes.bfloat16
    adj_agg = _softmax_rows_f32(adj.sum(axis=0, dtype=np.float32))
    adjT = np.ascontiguousarray(adj_agg.T)  # [j, i]
    hT = np.ascontiguousarray(h.T)          # [f, i]
    # ht2[p, b, i] = h.T[b*128+p, i]
    ht2 = np.ascontiguousarray(
        hT.reshape(2, 128, E).transpose(1, 0, 2)
    ).astype(bf16)
    ident = np.eye(128, dtype=bf16)
    in_maps = []
    for c in range(NCORES):
        js = slice(c * JPC, (c + 1) * JPC)
        # htc2[p, jl, b] = h[jg, b*128+p]
        htc2 = np.ascontiguousarray(
            h[js, :].reshape(JPC, 2, 128).transpose(2, 0, 1)
        )
        in_maps.append({
            "ht2": ht2,
            "htc2": htc2,
            "adjtb": np.ascontiguousarray(adjT[js, :]).astype(bf16),
            "ident": ident,
        })
    return in_maps


def kernel(h, adj, lin_w, lin_b):
    global LAST_RESULTS, LAST_RUN_WALL_S
    from concourse.bass_utils import run_bass_kernel_spmd

    h = np.asarray(h, dtype=np.float32)
    adj = np.asarray(adj, dtype=np.float32)

    nc = get_nc()
    in_maps = make_in_maps(h, adj)

    t0 = time.perf_counter()
    res = run_bass_kernel_spmd(nc, in_maps, core_ids=list(range(NCORES)))
    LAST_RUN_WALL_S = time.perf_counter() - t0
    LAST_RESULTS = res

    hpT = np.zeros((128, 2, E), dtype=np.float32)
    for c in range(NCORES):
        hpT += res.results[c]["hp"]
    # hpT[p, b, i] -> h_primeT[f, i] -> h_prime[i, f]
    h_prime = hpT.transpose(1, 0, 2).reshape(F, E).T
    out1 = np.where(h_prime > 0, h_prime, np.expm1(h_prime)).astype(np.float32)
    # alpha = softmax of a constant vector (see module docstring) -> uniform.
    alpha = np.full((1, E, 1), 1.0 / E, dtype=np.float32)
    return out1, alpha


# revision 32
# speedup vs baseline: 1.0057x; 1.0057x over previous
"""Trainium2 Bass kernel for nn_CenterNeighAtt (gnn_message_passing).

Reference computation (E=512, F=256, R=4):
    adj_agg   = softmax(adj.sum(0), axis=1)                       # [E,E]
    t[i,j,f]  = leaky_relu(adj_agg[i,j] * h[i,f] * h[j,f], 0.2)   # [E,E,F]
    att       = softmax(t, axis=0)  (over i)                      # [E,E,F]
    scores    = einsum('ijf,f->ij', att, w) + b
    alpha     = softmax(scores.sum(0)[None,:,None], axis=1)       # [1,E,1]
    h_prime   = einsum('ijf,jf->if', att, h)                      # [E,F]
    returns (elu(h_prime), alpha)

Key identities used:
  * sum_i att[i,j,f] == 1 exactly, so scores.sum(0)[j] = sum(w) + E*b is a
    constant vector -> alpha == uniform 1/E for ANY w, b. (Float deviation of
    the reference from uniform is ~1e-6 relative; far below tolerance.)
  * The softmax over i never needs the max-subtraction here: |t| <= ~2
    (adj_agg in (0, ~0.1), |h| <= ~5), so exp() is exact-safe in fp32/bf16.

Sharding: the j axis (neighbor axis) is sharded across the 8 NeuronCores,
64 j's per core. Each core computes, for its j-shard, the full [F, E(i)]
contribution sum_j att[i,j,f] * h[j,f] with NO collectives: softmax over i
is local because i lives entirely in the free axis. The host sums the 8
partial [F,E] outputs, transposes, applies elu.

Per-core layout: partitions = f (2 blocks of 128 paired in the free dim),
free = i (512), loop j. Per j (b = f-block index):
    DMA : Bt       = adj_agg[:,j] row broadcast to [128,E] bf16 (stride-0
                     partition source; DMA engines are otherwise idle)
    DVE : HH[b]    = ht2[b] * h[j,fb]              tensor_scalar, bf16 4x
    DVE : W2       = HH * Bt                       [128,2E] bf16 TT 2x
    DVE/ACT (rotated): L2 = leaky_relu(W2)         STT pair / Prelu pair
    ACT : u[b]     = exp(L2[b]), s[b] = rowsum     accum_out fused, bf16 out
    DVE : g2       = (1/s) * h[j,f]                tiny [128,2] ops
    DVE : diag_b   = I * g2[:,b]                   [128,128] bf16 diag matrix
    PE  : HP[b]   += diag_b.T @ u[b]               PSUM accumulation over j
j's are processed in pairs ([128, 2(j), 2(b), E] tiles) to halve per-op
overhead on the big passes. The TensorEngine's only job is the h_prime
accumulation (PSUM accumulate), the ScalarEngine does exp (+fused row-sums)
and two fifths of the leaky-relus, the VectorEngine does the bf16 product
chain and the rest.
"""

import time

import numpy as np

E, F, R, NCORES = 512, 256, 4, 8
JPC = E // NCORES  # j's per core
NEG_SLOPE = 0.2
# leaky relu runs on DVE for 3 of every 5 j-pairs, ACT for the rest
# (measured balance point of the two engines)

_CACHE = {}
LAST_RESULTS = None      # BassKernelResults of the last run (for test harness)
LAST_RUN_WALL_S = None   # wall time of the last run_bass_kernel_spmd call


def _build_nc():
    import concourse.bacc as bacc
    import concourse.mybir as mybir
    from concourse import tile

    f32 = mybir.dt.float32
    bf16 = mybir.dt.bfloat16
    Alu = mybir.AluOpType
    Act = mybir.ActivationFunctionType

    nc = bacc.Bacc(
        "TRN2", target_bir_lowering=False, debug=False, num_devices=NCORES
    )
    # ht2[p, b, i]  = h[i, b*128+p]   (both f-blocks side by side in free)
    ht2_d = nc.dram_tensor("ht2", [128, 2, E], bf16, kind="ExternalInput")
    # htc2[p, jl, b] = h[jg, b*128+p] (this core's j columns, per f-block)
    htc2_d = nc.dram_tensor("htc2", [128, JPC, 2], f32, kind="ExternalInput")
    # adjtb[jl, i] = adj_agg[i, jg] (bf16 rows, DMA-broadcast per j)
    adjtb_d = nc.dram_tensor("adjtb", [JPC, E], bf16, kind="ExternalInput")
    ident_d = nc.dram_tensor("ident", [128, 128], bf16, kind="ExternalInput")
    hp_d = nc.dram_tensor("hp", [128, 2, E], f32, kind="ExternalOutput")

    with tile.TileContext(nc) as tc:
        with (
            tc.tile_pool(name="const", bufs=1) as cpool,
            tc.tile_pool(name="bcast", bufs=6) as bpool,
            tc.tile_pool(name="wrk", bufs=5) as wpool,
            tc.tile_pool(name="lu", bufs=6) as lupool,
            tc.tile_pool(name="small", bufs=12) as spool,
            tc.tile_pool(name="psH", bufs=1, space="PSUM") as hpool,
        ):
            ht2 = cpool.tile([128, 2, E], bf16, tag="ht2")
            nc.sync.dma_start(ht2[:], ht2_d[:])
            htc2 = cpool.tile([128, JPC, 2], f32, tag="htc2")
            nc.sync.dma_start(htc2[:], htc2_d[:])
            ident = cpool.tile([128, 128], bf16, tag="ident")
            nc.sync.dma_start(ident[:], ident_d[:])

            HP = []
            for b in range(2):
                hp_acc = hpool.tile([128, E], f32, tag=f"hp{b}")
                HP.append(hp_acc)

            # process j's in pairs: [128, 2(j), 2(b), E] tiles halve the
            # per-op overhead tax on the big DVE/ACT passes
            NJP = JPC // 2
            for jp in range(NJP):
                j0 = 2 * jp
                bt2 = bpool.tile([128, 2, E], bf16, tag="bt2")
                nc.sync.dma_start(
                    bt2[:],
                    adjtb_d[j0:j0 + 2, :]
                    .rearrange("(o j) i -> o j i", o=1)
                    .broadcast_to((128, 2, E)),
                )

                hh4 = wpool.tile([128, 2, 2, E], bf16, tag="hh4")
                for jj in range(2):
                    for b in range(2):
                        nc.vector.tensor_scalar(
                            hh4[:, jj, b, :], ht2[:, b, :],
                            htc2[:, j0 + jj, b:b + 1], None, op0=Alu.mult,
                        )

                w4 = wpool.tile([128, 2, 2, E], bf16, tag="w4")
                nc.vector.tensor_tensor(
                    w4[:], hh4[:],
                    bt2[:].rearrange("p j (o i) -> p j o i", o=1)
                          .broadcast_to((128, 2, 2, E)),
                    op=Alu.mult,
                )

                # leaky relu, alternating DVE/ACT pairs to balance budgets
                l4 = wpool.tile([128, 2, 2, E], bf16, tag="l4")
                if jp % 5 < 3:
                    nc.vector.scalar_tensor_tensor(
                        l4[:], w4[:], NEG_SLOPE, w4[:],
                        op0=Alu.mult, op1=Alu.max,
                    )
                else:
                    nc.scalar.activation(l4[:], w4[:], Act.Prelu,
                                         alpha=NEG_SLOPE)

                # exp per (j, b) with the row-sum fused into the activation
                u4 = lupool.tile([128, 2, 2, E], bf16, tag="u4")
                s4 = spool.tile([128, 2, 2], f32, tag="s4")
                for jj in range(2):
                    for b in range(2):
                        nc.scalar.activation(
                            u4[:, jj, b, :], l4[:, jj, b, :], Act.Exp,
                            accum_out=s4[:, jj, b:b + 1],
                        )

                r4 = spool.tile([128, 2, 2], f32, tag="r4")
                nc.vector.reciprocal(r4[:], s4[:])
                g4 = spool.tile([128, 2, 2], f32, tag="g4")
                nc.vector.tensor_tensor(
                    g4[:], r4[:], htc2[:, j0:j0 + 2, :], op=Alu.mult
                )

                for jj in range(2):
                    for b in range(2):
                        diag = spool.tile([128, 128], bf16, tag=f"diag{b}")
                        nc.vector.tensor_scalar(
                            diag[:], ident[:], g4[:, jj, b:b + 1], None,
                            op0=Alu.mult,
                        )
                        nc.tensor.matmul(
                            HP[b][:],
                            diag[:],
                            u4[:, jj, b, :],
                            start=(jp == 0 and jj == 0),
                            stop=(jp == NJP - 1 and jj == 1),
                        )

            for b in range(2):
                hp_sb = wpool.tile([128, E], f32, tag=f"hpsb{b}")
                nc.vector.tensor_copy(hp_sb[:], HP[b][:])
                nc.sync.dma_start(hp_d[:, b, :], hp_sb[:])

    nc.compile()
    return nc


def get_nc():
    if "nc" not in _CACHE:
        _CACHE["nc"] = _build_nc()
    return _CACHE["nc"]


def _softmax_rows_f32(x):
    x = np.asarray(x, dtype=np.float32)
    m = x.max(axis=1, keepdims=True)
    e = np.exp(x - m, dtype=np.float32)
    return e / e.sum(axis=1, keepdims=True, dtype=np.float32)


def make_in_maps(h, adj):
    """Shard inputs for the 8 cores (host-side slicing/layout only)."""
    import ml_dtypes

    bf16 = ml_dtypes.bfloat16
    adj_agg = _softmax_rows_f32(adj.sum(axis=0, dtype=np.float32))
    adjT = np.ascontiguousarray(adj_agg.T)  # [j, i]
    hT = np.ascontiguousarray(h.T)          # [f, i]
    # ht2[p, b, i] = h.T[b*128+p, i]
    ht2 = np.ascontiguousarray(
        hT.reshape(2, 128, E).transpose(1, 0, 2)
    ).astype(bf16)
    ident = np.eye(128, dtype=bf16)
    in_maps = []
    for c in range(NCORES):
        js = slice(c * JPC, (c + 1) * JPC)
        # htc2[p, jl, b] = h[jg, b*128+p]
        htc2 = np.ascontiguousarray(
            h[js, :].reshape(JPC, 2, 128).transpose(2, 0, 1)
        )
        in_maps.append({
            "ht2": ht2,
            "htc2": htc2,
            "adjtb": np.ascontiguousarray(adjT[js, :]).astype(bf16),
            "ident": ident,
        })
    return in_maps


def kernel(h, adj, lin_w, lin_b):
    global LAST_RESULTS, LAST_RUN_WALL_S
    from concourse.bass_utils import run_bass_kernel_spmd

    h = np.asarray(h, dtype=np.float32)
    adj = np.asarray(adj, dtype=np.float32)

    nc = get_nc()
    in_maps = make_in_maps(h, adj)

    t0 = time.perf_counter()
    res = run_bass_kernel_spmd(nc, in_maps, core_ids=list(range(NCORES)))
    LAST_RUN_WALL_S = time.perf_counter() - t0
    LAST_RESULTS = res

    hpT = np.zeros((128, 2, E), dtype=np.float32)
    for c in range(NCORES):
        hpT += res.results[c]["hp"]
    # hpT[p, b, i] -> h_primeT[f, i] -> h_prime[i, f]
    h_prime = hpT.transpose(1, 0, 2).reshape(F, E).T
    out1 = np.where(h_prime > 0, h_prime, np.expm1(h_prime)).astype(np.float32)
    # alpha = softmax of a constant vector (see module docstring) -> uniform.
    alpha = np.full((1, E, 1), 1.0 / E, dtype=np.float32)
    return out1, alpha


# revision 41
# speedup vs baseline: 1.0076x; 1.0019x over previous
"""Trainium2 Bass kernel for nn_CenterNeighAtt (gnn_message_passing).

Reference computation (E=512, F=256, R=4):
    adj_agg   = softmax(adj.sum(0), axis=1)                       # [E,E]
    t[i,j,f]  = leaky_relu(adj_agg[i,j] * h[i,f] * h[j,f], 0.2)   # [E,E,F]
    att       = softmax(t, axis=0)  (over i)                      # [E,E,F]
    scores    = einsum('ijf,f->ij', att, w) + b
    alpha     = softmax(scores.sum(0)[None,:,None], axis=1)       # [1,E,1]
    h_prime   = einsum('ijf,jf->if', att, h)                      # [E,F]
    returns (elu(h_prime), alpha)

Key identities used:
  * sum_i att[i,j,f] == 1 exactly, so scores.sum(0)[j] = sum(w) + E*b is a
    constant vector -> alpha == uniform 1/E for ANY w, b. (Float deviation of
    the reference from uniform is ~1e-6 relative; far below tolerance.)
  * The softmax over i never needs the max-subtraction here: |t| <= ~2
    (adj_agg in (0, ~0.1), |h| <= ~5), so exp() is exact-safe in fp32/bf16.

Sharding: the j axis (neighbor axis) is sharded across the 8 NeuronCores,
64 j's per core. Each core computes, for its j-shard, the full [F, E(i)]
contribution sum_j att[i,j,f] * h[j,f] with NO collectives: softmax over i
is local because i lives entirely in the free axis. The host sums the 8
partial [F,E] outputs, transposes, applies elu.

Per-core layout: partitions = f (2 blocks of 128 paired in the free dim),
free = i (512), loop j. Per j (b = f-block index):
    DMA : Bt       = adj_agg[:,j] row broadcast to [128,E] bf16 (stride-0
                     partition source; DMA engines are otherwise idle)
    DVE : HH[b]    = ht2[b] * h[j,fb]              tensor_scalar, bf16 4x
    DVE : W2       = HH * Bt                       [128,2E] bf16 TT 2x
    DVE/ACT (rotated): L2 = leaky_relu(W2)         STT pair / Prelu pair
    ACT : u[b]     = exp(L2[b]), s[b] = rowsum     accum_out fused, bf16 out
    DVE : g2       = (1/s) * h[j,f]                tiny [128,2] ops
    DVE : diag_b   = I * g2[:,b]                   [128,128] bf16 diag matrix
    PE  : HP[b]   += diag_b.T @ u[b]               PSUM accumulation over j
j's are processed in pairs ([128, 2(j), 2(b), E] tiles) to halve per-op
overhead on the big passes. The TensorEngine's only job is the h_prime
accumulation (PSUM accumulate), the ScalarEngine does exp (+fused row-sums)
and two fifths of the leaky-relus, the VectorEngine does the bf16 product
chain and the rest.
"""

import time

import numpy as np

E, F, R, NCORES = 512, 256, 4, 8
JPC = E // NCORES  # j's per core
NEG_SLOPE = 0.2
# leaky relu runs on DVE for 3 of every 5 j-pairs, ACT for the rest
# (measured balance point of the two engines)

_CACHE = {}
LAST_RESULTS = None      # BassKernelResults of the last run (for test harness)
LAST_RUN_WALL_S = None   # wall time of the last run_bass_kernel_spmd call


def _build_nc():
    import concourse.bacc as bacc
    import concourse.mybir as mybir
    from concourse import tile

    f32 = mybir.dt.float32
    bf16 = mybir.dt.bfloat16
    Alu = mybir.AluOpType
    Act = mybir.ActivationFunctionType

    nc = bacc.Bacc(
        "TRN2", target_bir_lowering=False, debug=False, num_devices=NCORES
    )
    # ht2[p, b, i]  = h[i, b*128+p]   (both f-blocks side by side in free)
    ht2_d = nc.dram_tensor("ht2", [128, 2, E], bf16, kind="ExternalInput")
    # htc2[p, jl, b] = h[jg, b*128+p] (this core's j columns, per f-block)
    htc2_d = nc.dram_tensor("htc2", [128, JPC, 2], f32, kind="ExternalInput")
    # adjtb[jl, i] = adj_agg[i, jg] (bf16 rows, DMA-broadcast per j)
    adjtb_d = nc.dram_tensor("adjtb", [JPC, E], bf16, kind="ExternalInput")
    ident_d = nc.dram_tensor("ident", [128, 128], bf16, kind="ExternalInput")
    hp_d = nc.dram_tensor("hp", [128, 2, E], f32, kind="ExternalOutput")

    with tile.TileContext(nc) as tc:
        with (
            tc.tile_pool(name="const", bufs=1) as cpool,
            tc.tile_pool(name="bcast", bufs=6) as bpool,
            tc.tile_pool(name="wrk", bufs=5) as wpool,
            tc.tile_pool(name="lu", bufs=6) as lupool,
            tc.tile_pool(name="small", bufs=12) as spool,
            tc.tile_pool(name="psH", bufs=1, space="PSUM") as hpool,
        ):
            ht2 = cpool.tile([128, 2, E], bf16, tag="ht2")
            nc.sync.dma_start(ht2[:], ht2_d[:])
            htc2 = cpool.tile([128, JPC, 2], f32, tag="htc2")
            nc.sync.dma_start(htc2[:], htc2_d[:])
            ident = cpool.tile([128, 128], bf16, tag="ident")
            nc.sync.dma_start(ident[:], ident_d[:])

            HP = []
            for b in range(2):
                hp_acc = hpool.tile([128, E], f32, tag=f"hp{b}")
                HP.append(hp_acc)

            # process j's in pairs: [128, 2(j), 2(b), E] tiles halve the
            # per-op overhead tax on the big DVE/ACT passes
            NJP = JPC // 2
            for jp in range(NJP):
                j0 = 2 * jp
                bt2 = bpool.tile([128, 2, E], bf16, tag="bt2")
                nc.sync.dma_start(
                    bt2[:],
                    adjtb_d[j0:j0 + 2, :]
                    .rearrange("(o j) i -> o j i", o=1)
                    .broadcast_to((128, 2, E)),
                )

                hh4 = wpool.tile([128, 2, 2, E], bf16, tag="hh4")
                for jj in range(2):
                    for b in range(2):
                        nc.vector.tensor_scalar(
                            hh4[:, jj, b, :], ht2[:, b, :],
                            htc2[:, j0 + jj, b:b + 1], None, op0=Alu.mult,
                        )

                w4 = wpool.tile([128, 2, 2, E], bf16, tag="w4")
                nc.vector.tensor_tensor(
                    w4[:], hh4[:],
                    bt2[:].rearrange("p j (o i) -> p j o i", o=1)
                          .broadcast_to((128, 2, 2, E)),
                    op=Alu.mult,
                )

                # leaky relu, alternating DVE/ACT pairs to balance budgets
                # (measured optimum: 3 of 5 pairs on DVE via the fused
                # scalar_tensor_tensor (w*0.2) max w form)
                l4 = wpool.tile([128, 2, 2, E], bf16, tag="l4")
                if jp % 5 < 3:
                    nc.vector.scalar_tensor_tensor(
                        l4[:], w4[:], NEG_SLOPE, w4[:],
                        op0=Alu.mult, op1=Alu.max,
                    )
                else:
                    nc.scalar.activation(l4[:], w4[:], Act.Prelu,
                                         alpha=NEG_SLOPE)

                # exp per (j, b) with the row-sum fused into the activation
                u4 = lupool.tile([128, 2, 2, E], bf16, tag="u4")
                s4 = spool.tile([128, 2, 2], f32, tag="s4")
                for jj in range(2):
                    for b in range(2):
                        nc.scalar.activation(
                            u4[:, jj, b, :], l4[:, jj, b, :], Act.Exp,
                            accum_out=s4[:, jj, b:b + 1],
                        )

                r4 = spool.tile([128, 2, 2], f32, tag="r4")
                nc.vector.reciprocal(r4[:], s4[:])
                g4 = spool.tile([128, 2, 2], f32, tag="g4")
                nc.vector.tensor_tensor(
                    g4[:], r4[:], htc2[:, j0:j0 + 2, :], op=Alu.mult
                )

                for jj in range(2):
                    for b in range(2):
                        diag = spool.tile([128, 128], bf16, tag=f"diag{b}")
                        nc.vector.tensor_scalar(
                            diag[:], ident[:], g4[:, jj, b:b + 1], None,
                            op0=Alu.mult,
                        )
                        nc.tensor.matmul(
                            HP[b][:],
                            diag[:],
                            u4[:, jj, b, :],
                            start=(jp == 0 and jj == 0),
                            stop=(jp == NJP - 1 and jj == 1),
                        )

            for b in range(2):
                hp_sb = wpool.tile([128, E], f32, tag=f"hpsb{b}")
                nc.vector.tensor_copy(hp_sb[:], HP[b][:])
                nc.sync.dma_start(hp_d[:, b, :], hp_sb[:])

    nc.compile()
    return nc


def get_nc():
    if "nc" not in _CACHE:
        _CACHE["nc"] = _build_nc()
    return _CACHE["nc"]


def _softmax_rows_f32(x):
    x = np.asarray(x, dtype=np.float32)
    m = x.max(axis=1, keepdims=True)
    e = np.exp(x - m, dtype=np.float32)
    return e / e.sum(axis=1, keepdims=True, dtype=np.float32)


def make_in_maps(h, adj):
    """Shard inputs for the 8 cores (host-side slicing/layout only)."""
    import ml_dtypes

    bf16 = ml_dtypes.bfloat16
    adj_agg = _softmax_rows_f32(adj.sum(axis=0, dtype=np.float32))
    adjT = np.ascontiguousarray(adj_agg.T)  # [j, i]
    hT = np.ascontiguousarray(h.T)          # [f, i]
    # ht2[p, b, i] = h.T[b*128+p, i]
    ht2 = np.ascontiguousarray(
        hT.reshape(2, 128, E).transpose(1, 0, 2)
    ).astype(bf16)
    ident = np.eye(128, dtype=bf16)
    in_maps = []
    for c in range(NCORES):
        js = slice(c * JPC, (c + 1) * JPC)
        # htc2[p, jl, b] = h[jg, b*128+p]
        htc2 = np.ascontiguousarray(
            h[js, :].reshape(JPC, 2, 128).transpose(2, 0, 1)
        )
        in_maps.append({
            "ht2": ht2,
            "htc2": htc2,
            "adjtb": np.ascontiguousarray(adjT[js, :]).astype(bf16),
            "ident": ident,
        })
    return in_maps


def kernel(h, adj, lin_w, lin_b):
    global LAST_RESULTS, LAST_RUN_WALL_S
    from concourse.bass_utils import run_bass_kernel_spmd

    h = np.asarray(h, dtype=np.float32)
    adj = np.asarray(adj, dtype=np.float32)

    nc = get_nc()
    in_maps = make_in_maps(h, adj)

    t0 = time.perf_counter()
    res = run_bass_kernel_spmd(nc, in_maps, core_ids=list(range(NCORES)))
    LAST_RUN_WALL_S = time.perf_counter() - t0
    LAST_RESULTS = res

    hpT = np.zeros((128, 2, E), dtype=np.float32)
    for c in range(NCORES):
        hpT += res.results[c]["hp"]
    # hpT[p, b, i] -> h_primeT[f, i] -> h_prime[i, f]
    h_prime = hpT.transpose(1, 0, 2).reshape(F, E).T
    out1 = np.where(h_prime > 0, h_prime, np.expm1(h_prime)).astype(np.float32)
    # alpha = softmax of a constant vector (see module docstring) -> uniform.
    alpha = np.full((1, E, 1), 1.0 / E, dtype=np.float32)
    return out1, alpha
